# revision 17
# baseline (speedup 1.0000x reference)
"""Trainium2 Bass kernel for nn_MultiHeadMemory (sparse_attention).

Sharding: head-parallel across 8 NeuronCores (1 head per core).
Per core (head h):
  k_pre = mem_h @ fk_w.T + fk_b          [n, 128]   (centered weights -> zero k-mean)
  K~    = exp(rs_k[n] * k_pre)           (softmax numerator; LN mean/shift cancels in softmax)
  Z[n]  = sum_k K~                       (via ACT accum_out)
  V     = rs_v[n] * max(k_pre_v, 0)      (LN+relu, one DVE tensor_scalar op)
  S^T   = K~^T @ q^T                     [n, b]  (PE transpose of K~ per tile)
  P^T   = exp(recipZ[n] * S^T)
  oT   += V^T-contract: sum_n V[n,v] P^T[n,b];  s += sum_n P^T
  xraw  = oT^T @ fx_slice^T              [b, 128]
Host: x = sum_h xraw_h / s_h + fx_b; LayerNorm; relu.

LN trick: project with k-centered weights/bias so mean_k(k_pre) == 0 exactly;
then var = sumsq/128 (one tensor_tensor_reduce per tile) and
rs = (var+eps)^-1/2 = exp(-0.5*ln(var+eps)) (Ln+Exp share one ACT table set).
"""

import os
import sys
from contextlib import ExitStack

os.environ.setdefault("MYCRO_LOCAL_CACHE", "1")
for _p in ("/opt/trn_rl_repo",):
    if _p not in sys.path:
        sys.path.insert(0, _p)

import numpy as np

import concourse.bass as bass
import concourse.bacc as bacc
import concourse.mybir as mybir
import concourse.tile as tile
from concourse import bass2jax

F32 = mybir.dt.float32
ALU = mybir.AluOpType
ACTF = mybir.ActivationFunctionType

EPS = 1e-5
HEADS = 8
N_TOTAL = 65536
D = 128          # mem_dim
KD = 128         # key_dim
VD = 128         # val_dim
B = 256          # batch
N_CORES = 8
CHUNK = 128      # n-slots per tile
GROUP = 4        # tiles per group = one PSUM bank of k_pre / v_pre


def build_program(n_total=N_TOTAL):
    nchunks = n_total // CHUNK
    ngroups = nchunks // GROUP
    nc = bacc.Bacc(
        "TRN2",
        target_bir_lowering=False,
        debug=False,
        enable_asserts=False,
        num_devices=N_CORES,
    )
    memT = nc.dram_tensor("memT", [D, n_total], F32, kind="ExternalInput").ap()
    kwT = nc.dram_tensor("kwT", [D, KD], F32, kind="ExternalInput").ap()
    vwT = nc.dram_tensor("vwT", [D, VD], F32, kind="ExternalInput").ap()
    bk4 = nc.dram_tensor("bk4", [1, 4 * KD], F32, kind="ExternalInput").ap()
    bv4 = nc.dram_tensor("bv4", [1, 4 * VD], F32, kind="ExternalInput").ap()
    qT = nc.dram_tensor("qT", [KD, B], F32, kind="ExternalInput").ap()
    fxT = nc.dram_tensor("fxT", [VD, 128], F32, kind="ExternalInput").ap()
    ident = nc.dram_tensor("ident", [128, 128], F32, kind="ExternalInput").ap()
    ones1 = nc.dram_tensor("ones1", [1, 128], F32, kind="ExternalInput").ap()
    onescol = nc.dram_tensor("onescol", [128, 1], F32, kind="ExternalInput").ap()
    czero_eps = nc.dram_tensor("czero_eps", [128, 2], F32, kind="ExternalInput").ap()
    x_part = nc.dram_tensor("x_part", [B, 128], F32, kind="ExternalOutput").ap()
    s_out = nc.dram_tensor("s_out", [1, B], F32, kind="ExternalOutput").ap()

    with tile.TileContext(nc) as tc:
        with ExitStack() as ctx:
            _body(ctx, tc, memT, kwT, vwT, bk4, bv4, qT, fxT, ident, ones1,
                  onescol, czero_eps, x_part, s_out, nchunks, ngroups)
    nc.compile()
    return nc


def _body(ctx, tc, memT, kwT, vwT, bk4, bv4, qT, fxT, ident, ones1, onescol,
          czero_eps, x_part, s_out, nchunks, ngroups):
    nc = tc.nc
    const = ctx.enter_context(tc.tile_pool(name="const", bufs=1))

    cze = const.tile([128, 2], F32, tag="cze")
    nc.sync.dma_start(cze[:], czero_eps)
    nc.const_aps.aps[(F32, 0.0)] = cze[:, 0:1]
    nc.const_aps.aps[(F32, EPS)] = cze[:, 1:2]

    def load_const(ap, shape):
        t = const.tile(shape, F32, tag=f"c{ap.tensor.name}")
        nc.sync.dma_start(t[:], ap)
        return t

    kwT_sb = load_const(kwT, [D, KD])
    vwT_sb = load_const(vwT, [D, VD])
    bk4_sb = load_const(bk4, [1, 4 * KD])
    bv4_sb = load_const(bv4, [1, 4 * VD])
    qT_sb = load_const(qT, [KD, B])
    fxT_sb = load_const(fxT, [VD, 128])
    id_sb = load_const(ident, [128, 128])
    ones1_sb = load_const(ones1, [1, 128])
    onescol_sb = load_const(onescol, [128, 1])

    mem_pool = ctx.enter_context(tc.tile_pool(name="mem", bufs=3))
    kpre_pool = ctx.enter_context(tc.tile_pool(name="kpre", bufs=2, space="PSUM"))
    vpre_pool = ctx.enter_context(tc.tile_pool(name="vpre", bufs=2, space="PSUM"))
    ktp_pool = ctx.enter_context(tc.tile_pool(name="ktp", bufs=1, space="PSUM"))
    sT_pool = ctx.enter_context(tc.tile_pool(name="sT", bufs=2, space="PSUM"))
    acc_pool = ctx.enter_context(tc.tile_pool(name="acc", bufs=1, space="PSUM"))
    stats_pool = ctx.enter_context(tc.tile_pool(name="stats", bufs=3))
    kt_pool = ctx.enter_context(tc.tile_pool(name="ktil", bufs=4))
    vt_pool = ctx.enter_context(tc.tile_pool(name="vtil", bufs=8))
    ktT_pool = ctx.enter_context(tc.tile_pool(name="ktT", bufs=4))
    pt_pool = ctx.enter_context(tc.tile_pool(name="pt", bufs=3))
    dump_pool = ctx.enter_context(tc.tile_pool(name="dump", bufs=2))
    tail_pool = ctx.enter_context(tc.tile_pool(name="tail", bufs=1))

    # oT accumulator [v,b] in cols 0:256; softmax denom s in [0:1, 256:512].
    # Pre-zeroed via DVE; all matmuls accumulate with start=False so the
    # per-bank has_written clearing of start=True never wipes the co-tenant.
    acc = acc_pool.tile([128, 512], F32)
    nc.vector.memset(acc[:], 0.0)
    last = nchunks - 1

    for g in range(ngroups):
        mem_sb = mem_pool.tile([D, GROUP * CHUNK], F32, tag="mem")
        nc.sync.dma_start(mem_sb[:], memT[:, g * GROUP * CHUNK:(g + 1) * GROUP * CHUNK])

        kpre = kpre_pool.tile([128, GROUP * KD], F32, tag="kpre")
        vpre = vpre_pool.tile([128, GROUP * VD], F32, tag="vpre")

        for c in range(GROUP):
            sl = slice(c * CHUNK, (c + 1) * CHUNK)
            # rank-1 bias init (start=True) then projection accumulates on top
            nc.tensor.matmul(kpre[:, sl], ones1_sb[:], bk4_sb[:, sl], start=True, stop=False)
            nc.tensor.matmul(kpre[:, sl], mem_sb[:, sl], kwT_sb[:], start=False, stop=True)
            nc.tensor.matmul(vpre[:, sl], ones1_sb[:], bv4_sb[:, sl], start=True, stop=False)
            nc.tensor.matmul(vpre[:, sl], mem_sb[:, sl], vwT_sb[:], start=False, stop=True)

        # LN variance via bn_stats (even/odd split halves) + batched combine:
        # 128*var = M2e + M2o + 32*(mu_e - mu_o)^2  (n_e = n_o = 64)
        stats = stats_pool.tile([128, 12 * GROUP], F32, tag="ssq")
        for c in range(GROUP):
            sl = slice(c * CHUNK, (c + 1) * CHUNK)
            nc.vector.bn_stats(stats[:, 6 * c:6 * c + 6], kpre[:, sl])
            nc.vector.bn_stats(
                stats[:, 6 * (GROUP + c):6 * (GROUP + c) + 6], vpre[:, sl])
        dmu = stats_pool.tile([128, 2 * GROUP], F32, tag="dmu")
        nc.vector.tensor_sub(dmu[:], stats[:, 1::6], stats[:, 4::6])
        m2 = stats_pool.tile([128, 2 * GROUP], F32, tag="m2")
        nc.vector.tensor_add(m2[:], stats[:, 2::6], stats[:, 5::6])
        d2s = stats_pool.tile([128, 2 * GROUP], F32, tag="d2s")
        nc.vector.tensor_mul(d2s[:], dmu[:], dmu[:])
        nc.vector.tensor_scalar(out=d2s[:], in0=d2s[:], scalar1=32.0, scalar2=None,
                                op0=ALU.mult)
        v128 = stats_pool.tile([128, 2 * GROUP], F32, tag="v128")
        nc.vector.tensor_add(v128[:], m2[:], d2s[:])

        # rs = (var+eps)^-0.5 = exp(-0.5 * ln(v128/128 + eps)); Ln+Exp share a table set
        lnv = stats_pool.tile([128, 2 * GROUP], F32, tag="lnv")
        nc.scalar.activation(lnv[:], v128[:], ACTF.Ln, bias=EPS, scale=1.0 / CHUNK)
        rskv = stats_pool.tile([128, 2 * GROUP], F32, tag="rskv")
        nc.scalar.activation(rskv[:], lnv[:], ACTF.Exp, bias=0.0, scale=-0.5)

        zcols = stats_pool.tile([128, GROUP], F32, tag="z")
        vts = []
        sTs = []
        for c in range(GROUP):
            sl = slice(c * CHUNK, (c + 1) * CHUNK)
            kt = kt_pool.tile([128, KD], F32, tag="ktil")
            nc.scalar.activation(kt[:], kpre[:, sl], ACTF.Exp, bias=0.0,
                                 scale=rskv[:, c:c + 1], accum_out=zcols[:, c:c + 1])
            vt = vt_pool.tile([128, VD], F32, tag="vtil")
            nc.vector.tensor_scalar(
                out=vt[:], in0=vpre[:, sl], scalar1=0.0,
                scalar2=rskv[:, GROUP + c:GROUP + c + 1], op0=ALU.max, op1=ALU.mult)
            vts.append(vt)

            if c == 0:
                ktp = ktp_pool.tile([128, 512], F32, tag="ktp", name="ktp")
            nc.tensor.transpose(ktp[:, sl], kt[:], id_sb[:])
            ktT = ktT_pool.tile([128, CHUNK], F32, tag="ktT")
            if c % 2 == 0:
                nc.vector.tensor_copy(ktT[:], ktp[:, sl])
            else:
                nc.scalar.copy(ktT[:], ktp[:, sl])

            if c % 2 == 0:
                sT = sT_pool.tile([128, 512], F32, tag="sT")
                sTs.append(sT)
            ssl = slice((c % 2) * B, (c % 2 + 1) * B)
            nc.tensor.matmul(sT[:, ssl], ktT[:], qT_sb[:], start=True, stop=True)

        rz = stats_pool.tile([128, GROUP], F32, tag="rz")
        nc.vector.reciprocal(rz[:], zcols[:])

        for c in range(GROUP):
            t = g * GROUP + c
            ssl = slice((c % 2) * B, (c % 2 + 1) * B)
            pt = pt_pool.tile([128, B], F32, tag="pt")
            nc.scalar.activation(pt[:], sTs[c // 2][:, ssl], ACTF.Exp, bias=0.0,
                                 scale=rz[:, c:c + 1])
            nc.tensor.matmul(acc[:, 0:B], vts[c][:], pt[:],
                             start=False, stop=(t == last), skip_group_check=True)
            nc.tensor.matmul(acc[0:1, B:2 * B], onescol_sb[:], pt[:],
                             start=False, stop=(t == last), skip_group_check=True)

    # tail: evict accumulators, final fx matmul, DMA out
    oT_sb = tail_pool.tile([128, B], F32, tag="oT")
    nc.scalar.copy(oT_sb[:], acc[:, 0:B])
    s_sb = tail_pool.tile([1, B], F32, tag="s")
    nc.vector.tensor_copy(s_sb[:], acc[0:1, B:2 * B])
    nc.sync.dma_start(s_out, s_sb[:])

    xraw = sT_pool.tile([128, 512], F32, tag="sT")
    nc.tensor.matmul(xraw[:, 0:128], oT_sb[:, 0:128], fxT_sb[:], start=True, stop=True)
    nc.tensor.matmul(xraw[:, 128:256], oT_sb[:, 128:256], fxT_sb[:], start=True, stop=True)
    xr_sb = tail_pool.tile([128, 256], F32, tag="xr")
    nc.scalar.copy(xr_sb[:], xraw[:, 0:256])
    nc.sync.dma_start(x_part[0:128, :], xr_sb[:, 0:128])
    nc.sync.dma_start(x_part[128:256, :], xr_sb[:, 128:256])


def _prep_host(inputs, n_total=N_TOTAL):
    q = np.asarray(inputs["q"], np.float32)
    mem = np.asarray(inputs["mem"], np.float32)
    fk_w = np.asarray(inputs["fk_w"], np.float64)
    fk_b = np.asarray(inputs["fk_b"], np.float64)
    fv_w = np.asarray(inputs["fv_w"], np.float64)
    fv_b = np.asarray(inputs["fv_b"], np.float64)
    fx_w = np.asarray(inputs["fx_w"], np.float32)

    kwc = fk_w - fk_w.mean(axis=0, keepdims=True)   # center over key_dim
    bkc = fk_b - fk_b.mean()
    vwc = fv_w - fv_w.mean(axis=0, keepdims=True)   # center over val_dim
    bvc = fv_b - fv_b.mean()

    shared = {
        "kwT": np.ascontiguousarray(kwc.T).astype(np.float32),
        "vwT": np.ascontiguousarray(vwc.T).astype(np.float32),
        "bk4": np.tile(bkc.astype(np.float32)[None, :], (1, 4)),
        "bv4": np.tile(bvc.astype(np.float32)[None, :], (1, 4)),
        "qT": np.ascontiguousarray(q.T),
        "ident": np.eye(128, dtype=np.float32),
        "ones1": np.ones((1, 128), np.float32),
        "onescol": np.ones((128, 1), np.float32),
        "czero_eps": np.tile(np.array([[0.0, EPS]], np.float32), (128, 1)),
    }
    in_maps = []
    for h in range(N_CORES):
        m = dict(shared)
        m["memT"] = np.ascontiguousarray(mem[h, :n_total, :].T)
        m["fxT"] = np.ascontiguousarray(fx_w[:, h * 128:(h + 1) * 128].T).astype(np.float32)
        in_maps.append(m)
    return in_maps


def _epilogue(inputs, results):
    fx_b = np.asarray(inputs["fx_b"], np.float32)
    nx_g = np.asarray(inputs["nx_g"], np.float32)
    nx_b = np.asarray(inputs["nx_b"], np.float32)
    x = np.zeros((B, 128), np.float32)
    for h in range(N_CORES):
        s = results[h]["s_out"].reshape(B)
        x += results[h]["x_part"] / s[:, None]
    x = x + fx_b
    mu = x.mean(axis=-1, keepdims=True)
    var = np.square(x - mu).mean(axis=-1, keepdims=True)
    x = (x - mu) / np.sqrt(var + EPS) * nx_g + nx_b
    return np.maximum(x, 0.0).astype(np.float32)


_program_cache = {}


def _get_program(n_total=N_TOTAL):
    if n_total not in _program_cache:
        _program_cache[n_total] = build_program(n_total)
    return _program_cache[n_total]


def _make_runner(nc):
    """Cached variant of bass2jax.run_bass_via_pjrt's multi-core path: build
    the jitted sharded executable once, reuse across calls."""
    import jax
    import jax.numpy as jnp
    from jax.sharding import Mesh, PartitionSpec
    from jax.experimental.shard_map import shard_map
    import concourse.mybir as mb

    bass2jax.install_neuronx_cc_hook()
    partition_name = nc.partition_id_tensor.name if nc.partition_id_tensor else None

    in_names, out_names, out_avals, zero_outs = [], [], [], []
    for alloc in nc.m.functions[0].allocations:
        if not isinstance(alloc, mb.MemoryLocationSet):
            continue
        name = alloc.memorylocations[0].name
        if alloc.kind == "ExternalInput":
            if name != partition_name:
                in_names.append(name)
        elif alloc.kind == "ExternalOutput":
            shape = tuple(alloc.tensor_shape)
            dtype = mb.dt.np(alloc.dtype)
            out_avals.append(jax.core.ShapedArray(shape, dtype))
            out_names.append(name)
            zero_outs.append(np.zeros(shape, dtype))
    n_params = len(in_names)
    n_outs = len(out_avals)
    all_in_names = list(in_names) + list(out_names)
    if partition_name is not None:
        all_in_names.append(partition_name)

    def _body(*args):
        operands = list(args)
        if partition_name is not None:
            operands.append(bass2jax.partition_id_tensor())
        outs = bass2jax._bass_exec_p.bind(
            *operands,
            out_avals=tuple(out_avals),
            in_names=tuple(all_in_names),
            out_names=tuple(out_names),
            lowering_input_output_aliases=(),
            sim_require_finite=True,
            sim_require_nnan=True,
            nc=nc,
        )
        return tuple(outs)

    devices = jax.devices()[:N_CORES]
    mesh = Mesh(np.asarray(devices), ("core",))
    in_specs = (PartitionSpec("core"),) * (n_params + n_outs)
    out_specs = (PartitionSpec("core"),) * n_outs
    sharded = jax.jit(
        shard_map(_body, mesh=mesh, in_specs=in_specs, out_specs=out_specs,
                  check_rep=False),
        keep_unused=True,
    )

    def run(in_maps):
        concat_in = [
            np.concatenate([np.asarray(in_maps[c][nm]) for c in range(N_CORES)], axis=0)
            for nm in in_names
        ]
        concat_zeros = [
            np.zeros((N_CORES * z.shape[0], *z.shape[1:]), z.dtype) for z in zero_outs
        ]
        out_arrs = sharded(*concat_in, *concat_zeros)
        return [
            {nm: np.asarray(out_arrs[i]).reshape(N_CORES, *out_avals[i].shape)[c]
             for i, nm in enumerate(out_names)}
            for c in range(N_CORES)
        ], (concat_in, concat_zeros, sharded)

    return run


_runner_cache = {}


def _get_runner(n_total=N_TOTAL):
    if n_total not in _runner_cache:
        _runner_cache[n_total] = _make_runner(_get_program(n_total))
    return _runner_cache[n_total]


def _check_assumptions(inputs):
    for name, want in (("nk_g", 1.0), ("nv_g", 1.0)):
        if not np.allclose(np.asarray(inputs[name]), want):
            return False
    for name in ("nk_b", "nv_b"):
        if not np.allclose(np.asarray(inputs[name]), 0.0):
            return False
    return True


def _kernel_numpy(inputs):
    # exact fallback (never expected to trigger with spec fills)
    def ln(x, g, b):
        mu = x.mean(-1, keepdims=True)
        var = np.square(x - mu).mean(-1, keepdims=True)
        return (x - mu) / np.sqrt(var + EPS) * g + b

    def softmax(x):
        m = x.max(-1, keepdims=True)
        e = np.exp(x - m)
        return e / e.sum(-1, keepdims=True)

    q = np.asarray(inputs["q"], np.float32)
    mem = np.asarray(inputs["mem"], np.float32)
    k = softmax(ln(np.einsum('hnd,kd->hnk', mem, inputs["fk_w"]) + inputs["fk_b"],
                   inputs["nk_g"], inputs["nk_b"]))
    v = np.maximum(ln(np.einsum('hnd,vd->hnv', mem, inputs["fv_w"]) + inputs["fv_b"],
                      inputs["nv_g"], inputs["nv_b"]), 0.0)
    a = np.einsum('bk,hnk->bhn', q, k)
    w = softmax(a)
    o = np.einsum('bhn,hnv->bhv', w, v)
    x = o.reshape(o.shape[0], -1) @ np.asarray(inputs["fx_w"]).T + inputs["fx_b"]
    return np.maximum(ln(x, inputs["nx_g"], inputs["nx_b"]), 0.0).astype(np.float32)


def _run(inputs, n_total=N_TOTAL):
    runner = _get_runner(n_total)
    in_maps = _prep_host(inputs, n_total)
    results, handles = runner(in_maps)
    return _epilogue(inputs, results), results, handles


def kernel(**inputs):
    if not _check_assumptions(inputs):
        return _kernel_numpy(inputs)
    out, _, _ = _run(inputs)
    return out


# revision 18
# speedup vs baseline: 1.6825x; 1.6825x over previous
"""Trainium2 Bass kernel for nn_MultiHeadMemory (sparse_attention).

Sharding: head-parallel across 8 NeuronCores (1 head per core).
Per core (head h):
  k_pre = mem_h @ fk_w.T + fk_b          [n, 128]   (centered weights -> zero k-mean)
  K~    = exp(rs_k[n] * k_pre)           (softmax numerator; LN mean/shift cancels in softmax)
  Z[n]  = sum_k K~                       (via ACT accum_out)
  V     = rs_v[n] * max(k_pre_v, 0)      (LN+relu, one DVE tensor_scalar op)
  S^T   = K~^T @ q^T                     [n, b]  (PE transpose of K~ per tile)
  P^T   = exp(recipZ[n] * S^T)
  oT   += V^T-contract: sum_n V[n,v] P^T[n,b];  s += sum_n P^T
  xraw  = oT^T @ fx_slice^T              [b, 128]
Host: x = sum_h xraw_h / s_h + fx_b; LayerNorm; relu.

LN trick: project with k-centered weights/bias so mean_k(k_pre) == 0 exactly;
then var = sumsq/128 (one tensor_tensor_reduce per tile) and
rs = (var+eps)^-1/2 = exp(-0.5*ln(var+eps)) (Ln+Exp share one ACT table set).
"""

import os
import sys
from contextlib import ExitStack

os.environ.setdefault("MYCRO_LOCAL_CACHE", "1")
for _p in ("/opt/trn_rl_repo",):
    if _p not in sys.path:
        sys.path.insert(0, _p)

import numpy as np

import concourse.bass as bass
import concourse.bacc as bacc
import concourse.mybir as mybir
import concourse.tile as tile
from concourse import bass2jax

F32 = mybir.dt.float32
ALU = mybir.AluOpType
ACTF = mybir.ActivationFunctionType

EPS = 1e-5
HEADS = 8
N_TOTAL = 65536
D = 128          # mem_dim
KD = 128         # key_dim
VD = 128         # val_dim
B = 256          # batch
N_CORES = 8
CHUNK = 128      # n-slots per tile
GROUP = 4        # tiles per group = one PSUM bank of k_pre / v_pre


def build_program(n_total=N_TOTAL):
    nchunks = n_total // CHUNK
    ngroups = nchunks // GROUP
    nc = bacc.Bacc(
        "TRN2",
        target_bir_lowering=False,
        debug=False,
        enable_asserts=False,
        num_devices=N_CORES,
    )
    memT = nc.dram_tensor("memT", [D, n_total], F32, kind="ExternalInput").ap()
    kwT = nc.dram_tensor("kwT", [D, KD], F32, kind="ExternalInput").ap()
    vwT = nc.dram_tensor("vwT", [D, VD], F32, kind="ExternalInput").ap()
    bk4 = nc.dram_tensor("bk4", [1, 4 * KD], F32, kind="ExternalInput").ap()
    bv4 = nc.dram_tensor("bv4", [1, 4 * VD], F32, kind="ExternalInput").ap()
    qT = nc.dram_tensor("qT", [KD, B], F32, kind="ExternalInput").ap()
    fxT = nc.dram_tensor("fxT", [VD, 128], F32, kind="ExternalInput").ap()
    ident = nc.dram_tensor("ident", [128, 128], F32, kind="ExternalInput").ap()
    ones1 = nc.dram_tensor("ones1", [1, 128], F32, kind="ExternalInput").ap()
    onescol = nc.dram_tensor("onescol", [128, 1], F32, kind="ExternalInput").ap()
    czero_eps = nc.dram_tensor("czero_eps", [128, 2], F32, kind="ExternalInput").ap()
    x_part = nc.dram_tensor("x_part", [B, 128], F32, kind="ExternalOutput").ap()
    s_out = nc.dram_tensor("s_out", [1, B], F32, kind="ExternalOutput").ap()

    with tile.TileContext(nc) as tc:
        with ExitStack() as ctx:
            _body(ctx, tc, memT, kwT, vwT, bk4, bv4, qT, fxT, ident, ones1,
                  onescol, czero_eps, x_part, s_out, nchunks, ngroups)
    nc.compile()
    return nc


def _body(ctx, tc, memT, kwT, vwT, bk4, bv4, qT, fxT, ident, ones1, onescol,
          czero_eps, x_part, s_out, nchunks, ngroups):
    nc = tc.nc
    const = ctx.enter_context(tc.tile_pool(name="const", bufs=1))

    cze = const.tile([128, 2], F32, tag="cze")
    nc.sync.dma_start(cze[:], czero_eps)
    nc.const_aps.aps[(F32, 0.0)] = cze[:, 0:1]
    nc.const_aps.aps[(F32, EPS)] = cze[:, 1:2]

    def load_const(ap, shape):
        t = const.tile(shape, F32, tag=f"c{ap.tensor.name}")
        nc.sync.dma_start(t[:], ap)
        return t

    kwT_sb = load_const(kwT, [D, KD])
    vwT_sb = load_const(vwT, [D, VD])
    bk4_sb = load_const(bk4, [1, 4 * KD])
    bv4_sb = load_const(bv4, [1, 4 * VD])
    qT_sb = load_const(qT, [KD, B])
    fxT_sb = load_const(fxT, [VD, 128])
    id_sb = load_const(ident, [128, 128])
    ones1_sb = load_const(ones1, [1, 128])
    onescol_sb = load_const(onescol, [128, 1])

    mem_pool = ctx.enter_context(tc.tile_pool(name="mem", bufs=3))
    kpre_pool = ctx.enter_context(tc.tile_pool(name="kpre", bufs=2, space="PSUM"))
    vpre_pool = ctx.enter_context(tc.tile_pool(name="vpre", bufs=2, space="PSUM"))
    ktp_pool = ctx.enter_context(tc.tile_pool(name="ktp", bufs=1, space="PSUM"))
    sT_pool = ctx.enter_context(tc.tile_pool(name="sT", bufs=2, space="PSUM"))
    acc_pool = ctx.enter_context(tc.tile_pool(name="acc", bufs=1, space="PSUM"))
    stats_pool = ctx.enter_context(tc.tile_pool(name="stats", bufs=3))
    kt_pool = ctx.enter_context(tc.tile_pool(name="ktil", bufs=4))
    vt_pool = ctx.enter_context(tc.tile_pool(name="vtil", bufs=8))
    ktT_pool = ctx.enter_context(tc.tile_pool(name="ktT", bufs=4))
    pt_pool = ctx.enter_context(tc.tile_pool(name="pt", bufs=3))
    dump_pool = ctx.enter_context(tc.tile_pool(name="dump", bufs=2))
    tail_pool = ctx.enter_context(tc.tile_pool(name="tail", bufs=1))

    # oT accumulator [v,b] in cols 0:256; softmax denom s in [0:1, 256:512].
    # Pre-zeroed via DVE; all matmuls accumulate with start=False so the
    # per-bank has_written clearing of start=True never wipes the co-tenant.
    acc = acc_pool.tile([128, 512], F32)
    nc.vector.memset(acc[:], 0.0)
    last = nchunks - 1

    for g in range(ngroups):
        mem_sb = mem_pool.tile([D, GROUP * CHUNK], F32, tag="mem")
        nc.sync.dma_start(mem_sb[:], memT[:, g * GROUP * CHUNK:(g + 1) * GROUP * CHUNK])

        kpre = kpre_pool.tile([128, GROUP * KD], F32, tag="kpre")
        vpre = vpre_pool.tile([128, GROUP * VD], F32, tag="vpre")

        # one rank-1 bias matmul per bank (start=True marks the whole bank);
        # projections accumulate on top, only the last chunk carries stop=True
        # so the sim's bank-granular group flag survives chunks 0..GROUP-2
        nc.tensor.matmul(kpre[:], ones1_sb[:], bk4_sb[:], start=True, stop=False)
        nc.tensor.matmul(vpre[:], ones1_sb[:], bv4_sb[:], start=True, stop=False)
        for c in range(GROUP):
            sl = slice(c * CHUNK, (c + 1) * CHUNK)
            lastc = c == GROUP - 1
            nc.tensor.matmul(kpre[:, sl], mem_sb[:, sl], kwT_sb[:], start=False, stop=lastc)
            nc.tensor.matmul(vpre[:, sl], mem_sb[:, sl], vwT_sb[:], start=False, stop=lastc)

        # LN variance via bn_stats (even/odd split halves) + batched combine:
        # 128*var = M2e + M2o + 32*(mu_e - mu_o)^2  (n_e = n_o = 64)
        stats = stats_pool.tile([128, 12 * GROUP], F32, tag="ssq")
        for c in range(GROUP):
            sl = slice(c * CHUNK, (c + 1) * CHUNK)
            nc.vector.bn_stats(stats[:, 6 * c:6 * c + 6], kpre[:, sl])
            nc.vector.bn_stats(
                stats[:, 6 * (GROUP + c):6 * (GROUP + c) + 6], vpre[:, sl])
        dmu = stats_pool.tile([128, 2 * GROUP], F32, tag="dmu")
        nc.vector.tensor_sub(dmu[:], stats[:, 1::6], stats[:, 4::6])
        m2 = stats_pool.tile([128, 2 * GROUP], F32, tag="m2")
        nc.vector.tensor_add(m2[:], stats[:, 2::6], stats[:, 5::6])
        d2s = stats_pool.tile([128, 2 * GROUP], F32, tag="d2s")
        nc.vector.tensor_mul(d2s[:], dmu[:], dmu[:])
        nc.vector.tensor_scalar(out=d2s[:], in0=d2s[:], scalar1=32.0, scalar2=None,
                                op0=ALU.mult)
        v128 = stats_pool.tile([128, 2 * GROUP], F32, tag="v128")
        nc.vector.tensor_add(v128[:], m2[:], d2s[:])

        # rs = (var+eps)^-0.5 = exp(-0.5 * ln(v128/128 + eps)); Ln+Exp share a table set
        lnv = stats_pool.tile([128, 2 * GROUP], F32, tag="lnv")
        nc.scalar.activation(lnv[:], v128[:], ACTF.Ln, bias=EPS, scale=1.0 / CHUNK)
        rskv = stats_pool.tile([128, 2 * GROUP], F32, tag="rskv")
        nc.scalar.activation(rskv[:], lnv[:], ACTF.Exp, bias=0.0, scale=-0.5)

        zcols = stats_pool.tile([128, GROUP], F32, tag="z")
        vts = []
        sTs = []
        for c in range(GROUP):
            sl = slice(c * CHUNK, (c + 1) * CHUNK)
            kt = kt_pool.tile([128, KD], F32, tag="ktil")
            nc.scalar.activation(kt[:], kpre[:, sl], ACTF.Exp, bias=0.0,
                                 scale=rskv[:, c:c + 1], accum_out=zcols[:, c:c + 1])
            vt = vt_pool.tile([128, VD], F32, tag="vtil")
            nc.vector.tensor_scalar(
                out=vt[:], in0=vpre[:, sl], scalar1=0.0,
                scalar2=rskv[:, GROUP + c:GROUP + c + 1], op0=ALU.max, op1=ALU.mult)
            vts.append(vt)

            if c == 0:
                ktp = ktp_pool.tile([128, 512], F32, tag="ktp", name="ktp")
            nc.tensor.transpose(ktp[:, sl], kt[:], id_sb[:])
            ktT = ktT_pool.tile([128, CHUNK], F32, tag="ktT")
            if c % 2 == 0:
                nc.vector.tensor_copy(ktT[:], ktp[:, sl])
            else:
                nc.scalar.copy(ktT[:], ktp[:, sl])

            if c % 2 == 0:
                sT = sT_pool.tile([128, 512], F32, tag="sT")
                sTs.append(sT)
            ssl = slice((c % 2) * B, (c % 2 + 1) * B)
            nc.tensor.matmul(sT[:, ssl], ktT[:], qT_sb[:], start=True, stop=True)

        rz = stats_pool.tile([128, GROUP], F32, tag="rz")
        nc.vector.reciprocal(rz[:], zcols[:])

        for c in range(GROUP):
            t = g * GROUP + c
            ssl = slice((c % 2) * B, (c % 2 + 1) * B)
            pt = pt_pool.tile([128, B], F32, tag="pt")
            nc.scalar.activation(pt[:], sTs[c // 2][:, ssl], ACTF.Exp, bias=0.0,
                                 scale=rz[:, c:c + 1])
            nc.tensor.matmul(acc[:, 0:B], vts[c][:], pt[:],
                             start=False, stop=(t == last), skip_group_check=True)
            nc.tensor.matmul(acc[0:1, B:2 * B], onescol_sb[:], pt[:],
                             start=False, stop=(t == last), skip_group_check=True)

    # tail: evict accumulators, final fx matmul, DMA out
    oT_sb = tail_pool.tile([128, B], F32, tag="oT")
    nc.scalar.copy(oT_sb[:], acc[:, 0:B])
    s_sb = tail_pool.tile([1, B], F32, tag="s")
    nc.vector.tensor_copy(s_sb[:], acc[0:1, B:2 * B])
    nc.sync.dma_start(s_out, s_sb[:])

    xraw = sT_pool.tile([128, 512], F32, tag="sT")
    nc.tensor.matmul(xraw[:, 0:128], oT_sb[:, 0:128], fxT_sb[:], start=True, stop=True)
    nc.tensor.matmul(xraw[:, 128:256], oT_sb[:, 128:256], fxT_sb[:], start=True, stop=True)
    xr_sb = tail_pool.tile([128, 256], F32, tag="xr")
    nc.scalar.copy(xr_sb[:], xraw[:, 0:256])
    nc.sync.dma_start(x_part[0:128, :], xr_sb[:, 0:128])
    nc.sync.dma_start(x_part[128:256, :], xr_sb[:, 128:256])


def _prep_host(inputs, n_total=N_TOTAL):
    q = np.asarray(inputs["q"], np.float32)
    mem = np.asarray(inputs["mem"], np.float32)
    fk_w = np.asarray(inputs["fk_w"], np.float64)
    fk_b = np.asarray(inputs["fk_b"], np.float64)
    fv_w = np.asarray(inputs["fv_w"], np.float64)
    fv_b = np.asarray(inputs["fv_b"], np.float64)
    fx_w = np.asarray(inputs["fx_w"], np.float32)

    kwc = fk_w - fk_w.mean(axis=0, keepdims=True)   # center over key_dim
    bkc = fk_b - fk_b.mean()
    vwc = fv_w - fv_w.mean(axis=0, keepdims=True)   # center over val_dim
    bvc = fv_b - fv_b.mean()

    shared = {
        "kwT": np.ascontiguousarray(kwc.T).astype(np.float32),
        "vwT": np.ascontiguousarray(vwc.T).astype(np.float32),
        "bk4": np.tile(bkc.astype(np.float32)[None, :], (1, 4)),
        "bv4": np.tile(bvc.astype(np.float32)[None, :], (1, 4)),
        "qT": np.ascontiguousarray(q.T),
        "ident": np.eye(128, dtype=np.float32),
        "ones1": np.ones((1, 128), np.float32),
        "onescol": np.ones((128, 1), np.float32),
        "czero_eps": np.tile(np.array([[0.0, EPS]], np.float32), (128, 1)),
    }
    in_maps = []
    for h in range(N_CORES):
        m = dict(shared)
        m["memT"] = np.ascontiguousarray(mem[h, :n_total, :].T)
        m["fxT"] = np.ascontiguousarray(fx_w[:, h * 128:(h + 1) * 128].T).astype(np.float32)
        in_maps.append(m)
    return in_maps


def _epilogue(inputs, results):
    fx_b = np.asarray(inputs["fx_b"], np.float32)
    nx_g = np.asarray(inputs["nx_g"], np.float32)
    nx_b = np.asarray(inputs["nx_b"], np.float32)
    x = np.zeros((B, 128), np.float32)
    for h in range(N_CORES):
        s = results[h]["s_out"].reshape(B)
        x += results[h]["x_part"] / s[:, None]
    x = x + fx_b
    mu = x.mean(axis=-1, keepdims=True)
    var = np.square(x - mu).mean(axis=-1, keepdims=True)
    x = (x - mu) / np.sqrt(var + EPS) * nx_g + nx_b
    return np.maximum(x, 0.0).astype(np.float32)


_program_cache = {}


def _get_program(n_total=N_TOTAL):
    if n_total not in _program_cache:
        _program_cache[n_total] = build_program(n_total)
    return _program_cache[n_total]


def _make_runner(nc):
    """Cached variant of bass2jax.run_bass_via_pjrt's multi-core path: build
    the jitted sharded executable once, reuse across calls."""
    import jax
    import jax.numpy as jnp
    from jax.sharding import Mesh, PartitionSpec
    from jax.experimental.shard_map import shard_map
    import concourse.mybir as mb

    bass2jax.install_neuronx_cc_hook()
    partition_name = nc.partition_id_tensor.name if nc.partition_id_tensor else None

    in_names, out_names, out_avals, zero_outs = [], [], [], []
    for alloc in nc.m.functions[0].allocations:
        if not isinstance(alloc, mb.MemoryLocationSet):
            continue
        name = alloc.memorylocations[0].name
        if alloc.kind == "ExternalInput":
            if name != partition_name:
                in_names.append(name)
        elif alloc.kind == "ExternalOutput":
            shape = tuple(alloc.tensor_shape)
            dtype = mb.dt.np(alloc.dtype)
            out_avals.append(jax.core.ShapedArray(shape, dtype))
            out_names.append(name)
            zero_outs.append(np.zeros(shape, dtype))
    n_params = len(in_names)
    n_outs = len(out_avals)
    all_in_names = list(in_names) + list(out_names)
    if partition_name is not None:
        all_in_names.append(partition_name)

    def _body(*args):
        operands = list(args)
        if partition_name is not None:
            operands.append(bass2jax.partition_id_tensor())
        outs = bass2jax._bass_exec_p.bind(
            *operands,
            out_avals=tuple(out_avals),
            in_names=tuple(all_in_names),
            out_names=tuple(out_names),
            lowering_input_output_aliases=(),
            sim_require_finite=True,
            sim_require_nnan=True,
            nc=nc,
        )
        return tuple(outs)

    devices = jax.devices()[:N_CORES]
    mesh = Mesh(np.asarray(devices), ("core",))
    in_specs = (PartitionSpec("core"),) * (n_params + n_outs)
    out_specs = (PartitionSpec("core"),) * n_outs
    sharded = jax.jit(
        shard_map(_body, mesh=mesh, in_specs=in_specs, out_specs=out_specs,
                  check_rep=False),
        keep_unused=True,
    )

    def run(in_maps):
        concat_in = [
            np.concatenate([np.asarray(in_maps[c][nm]) for c in range(N_CORES)], axis=0)
            for nm in in_names
        ]
        concat_zeros = [
            np.zeros((N_CORES * z.shape[0], *z.shape[1:]), z.dtype) for z in zero_outs
        ]
        out_arrs = sharded(*concat_in, *concat_zeros)
        return [
            {nm: np.asarray(out_arrs[i]).reshape(N_CORES, *out_avals[i].shape)[c]
             for i, nm in enumerate(out_names)}
            for c in range(N_CORES)
        ], (concat_in, concat_zeros, sharded)

    return run


_runner_cache = {}


def _get_runner(n_total=N_TOTAL):
    if n_total not in _runner_cache:
        _runner_cache[n_total] = _make_runner(_get_program(n_total))
    return _runner_cache[n_total]


def _check_assumptions(inputs):
    for name, want in (("nk_g", 1.0), ("nv_g", 1.0)):
        if not np.allclose(np.asarray(inputs[name]), want):
            return False
    for name in ("nk_b", "nv_b"):
        if not np.allclose(np.asarray(inputs[name]), 0.0):
            return False
    return True


def _kernel_numpy(inputs):
    # exact fallback (never expected to trigger with spec fills)
    def ln(x, g, b):
        mu = x.mean(-1, keepdims=True)
        var = np.square(x - mu).mean(-1, keepdims=True)
        return (x - mu) / np.sqrt(var + EPS) * g + b

    def softmax(x):
        m = x.max(-1, keepdims=True)
        e = np.exp(x - m)
        return e / e.sum(-1, keepdims=True)

    q = np.asarray(inputs["q"], np.float32)
    mem = np.asarray(inputs["mem"], np.float32)
    k = softmax(ln(np.einsum('hnd,kd->hnk', mem, inputs["fk_w"]) + inputs["fk_b"],
                   inputs["nk_g"], inputs["nk_b"]))
    v = np.maximum(ln(np.einsum('hnd,vd->hnv', mem, inputs["fv_w"]) + inputs["fv_b"],
                      inputs["nv_g"], inputs["nv_b"]), 0.0)
    a = np.einsum('bk,hnk->bhn', q, k)
    w = softmax(a)
    o = np.einsum('bhn,hnv->bhv', w, v)
    x = o.reshape(o.shape[0], -1) @ np.asarray(inputs["fx_w"]).T + inputs["fx_b"]
    return np.maximum(ln(x, inputs["nx_g"], inputs["nx_b"]), 0.0).astype(np.float32)


def _run(inputs, n_total=N_TOTAL):
    runner = _get_runner(n_total)
    in_maps = _prep_host(inputs, n_total)
    results, handles = runner(in_maps)
    return _epilogue(inputs, results), results, handles


def kernel(**inputs):
    if not _check_assumptions(inputs):
        return _kernel_numpy(inputs)
    out, _, _ = _run(inputs)
    return out
